# revision 27
# baseline (speedup 1.0000x reference)
"""Distributed Trainium2 Bass kernel for GQA causal attention with RoPE.

Problem: B=2, S=2048, DIM=2048, 32 Q heads, 8 KV heads (GQA 4:1), hd=64,
causal, rotary embeddings, fp32 in/out.

Sharding over 8 cores: data-parallel over batch (2) x tensor-parallel over
KV-head groups (4 groups of 2 KV heads, each with its 8 Q heads).
Core c: batch b = c // 4, group g = c % 4.  Each core computes a partial
output projection; the host sums the 4 partials per batch.

v2 design (vs baseline): everything except PSUM accumulation runs in bf16
(x/wq/wo cast host-side).  QKV is computed per 1024-token chunk; attention
per 512-token subchunk with fine-grained causal narrowing on the diagonal
128-tiles.  The PE instruction stream is kept dense by interleaving
independent matmuls (output projection of the previous subchunk, QKV of the
next chunk) into the Act-coupled attention loop, which keeps the HAM clock
gate at full rate.  Causal masking is a multiplicative bf16 lower-tri mask
on the DVE (post-exp); softmax denominators come free from a ones-column in
the V operand and are inverted per-block with reciprocal_approx_fast.

Layout tricks (kept from baseline):
- head_dim permuted even-first (via Wq/Wk row permutation) so RoPE becomes
  32-row block ops; local Q heads paired (l, l+4) per 128-row tile so the
  natural K tile provides the score stationary operand for both pair
  members; woT rows reordered to match.
"""
import os
import sys

if "/opt/trn_rl_repo" not in sys.path:
    sys.path.insert(0, "/opt/trn_rl_repo")

import contextlib

import numpy as np

import concourse.bass as bass
import concourse.tile as tile
from concourse import bacc, mybir
from concourse import bass_utils
from concourse.masks import make_identity

F32 = mybir.dt.float32
BF16 = mybir.dt.bfloat16
EXP = mybir.ActivationFunctionType.Exp
COPYF = mybir.ActivationFunctionType.Copy

B, S, D = 2, 2048, 2048
NH, NKV, HD = 32, 8, 64
HL = 8           # local Q heads per core
KVL = 2          # local KV heads per core
EQ = HL * HD     # 512 local q features
EK = KVL * HD    # 128
EV = KVL * HD    # 128
EQKV = EQ + EK + EV  # 768
NT = S // 128    # 16 token tiles
NSC = S // 512   # 4 subchunks (attention granularity)
NC = S // 1024   # 2 chunks (QKV granularity)
SCALE = 1.0 / 8.0

_CACHED_NC = None


def build():
    nc = bacc.Bacc("TRN2", target_bir_lowering=False, debug=False)
    # x is host-transposed: [D, S], bf16
    x_d = nc.dram_tensor("x", [D, S], BF16, kind="ExternalInput").ap()
    wq_d = nc.dram_tensor("wq", [D, EQKV], BF16, kind="ExternalInput").ap()
    wo_d = nc.dram_tensor("wo", [EQ, D], BF16, kind="ExternalInput").ap()
    # rope rows: 0:128 = cos x4, 128:256 = [+sin, -sin, +sin, -sin]
    # (sin signs placed so each t1 mul reads t0 and sin at the SAME base
    # partition -- walrus requires equal base partitions for two SB inputs)
    rope_d = nc.dram_tensor("rope", [256, S], BF16, kind="ExternalInput").ap()
    out_d = nc.dram_tensor("out", [S, D], F32, kind="ExternalOutput").ap()
    DBG = bool(int(os.environ.get("K_DEBUG", "0")))
    if DBG:
        dbg_q = nc.dram_tensor("dbg_q", [128, S], BF16,
                               kind="ExternalOutput").ap()
        dbg_k = nc.dram_tensor("dbg_k", [128, S], BF16,
                               kind="ExternalOutput").ap()
        dbg_v = nc.dram_tensor("dbg_v", [128, 130], BF16,
                               kind="ExternalOutput").ap()
        dbg_ao = nc.dram_tensor("dbg_ao", [128, S], BF16,
                                kind="ExternalOutput").ap()
        dbg_dn = nc.dram_tensor("dbg_dn", [8, 512], F32,
                                kind="ExternalOutput").ap()

    with tile.TileContext(nc) as tc:
        ctx = contextlib.ExitStack()
        with ctx:
            const = ctx.enter_context(tc.tile_pool(name="const", bufs=1))
            persist = ctx.enter_context(tc.tile_pool(name="persist", bufs=1))
            xtp = ctx.enter_context(tc.tile_pool(name="xt", bufs=16))
            ropep = ctx.enter_context(tc.tile_pool(name="ropep", bufs=3))
            pbfp = ctx.enter_context(tc.tile_pool(name="pbf", bufs=4))
            dnp = ctx.enter_context(tc.tile_pool(name="dnp", bufs=3))
            rbp = ctx.enter_context(tc.tile_pool(name="rbp", bufs=3))
            ysbp = ctx.enter_context(tc.tile_pool(name="ysb", bufs=3))
            # PSUM (8 banks): psq 1x2, ss 2x1, po 2x1, psy 1x(psy+pt) = 8
            ps_q = ctx.enter_context(
                tc.tile_pool(name="psq", bufs=1, space="PSUM"))
            ps_s = ctx.enter_context(
                tc.tile_pool(name="pss", bufs=2, space="PSUM"))
            ps_o = ctx.enter_context(
                tc.tile_pool(name="pso", bufs=2, space="PSUM"))
            ps_y = ctx.enter_context(
                tc.tile_pool(name="psy", bufs=1, space="PSUM"))

            # ---- constants ----
            ident = const.tile([128, 128], BF16)
            make_identity(nc, ident[:])
            # lower-tri (keep q>=k) multiplicative mask: tri[p,j]=1 iff j>=p
            ones_t = const.tile([128, 128], BF16, name="ones_t")
            nc.gpsimd.memset(ones_t[:], 1.0)
            tri = const.tile([128, 128], BF16, name="tri")
            nc.gpsimd.affine_select(
                out=tri[:], in_=ones_t[:],
                compare_op=mybir.AluOpType.is_ge,
                fill=0.0, base=0, channel_multiplier=-1,
                pattern=[[1, 128]])

            # ---- weights + x loads, issue-interleaved so the first QKV
            # dt-chain starts as early as possible (SP issues ~645ns/DMA) ----
            wq_sb, xT = [], []
            for dt in range(16):
                wb = const.tile([128, EQKV], BF16, tag=f"wqb{dt}",
                                name=f"wqb{dt}")
                nc.sync.dma_start(wb[:], wq_d[128 * dt:128 * (dt + 1), :])
                wq_sb.append(wb)
                xt = xtp.tile([128, S], BF16, tag="xt", name=f"xT{dt}")
                # x issued from the Act DGE queue, in parallel with wq on SP
                nc.scalar.dma_start(xt[:], x_d[128 * dt:128 * (dt + 1), :])
                xT.append(xt)
            cosF = const.tile([128, S], BF16)
            nc.sync.dma_start(cosF[:], rope_d[0:128, :])
            sinF = const.tile([128, S], BF16)
            nc.sync.dma_start(sinF[:], rope_d[128:256, :])
            wo_bf = []
            for dt in range(4):
                wb = const.tile([128, D], BF16, tag=f"wob{dt}", name=f"wob{dt}")
                nc.sync.dma_start(wb[:], wo_d[128 * dt:128 * (dt + 1), :])
                wo_bf.append(wb)

            # ---- persistent activation buffers ----
            qT = [persist.tile([128, S], BF16, tag=f"qT{i}", name=f"qT{i}")
                  for i in range(4)]
            kT = persist.tile([128, S], BF16, name="kT")
            v_aug = [persist.tile([128, 130], BF16, tag=f"vaug{i}",
                                  name=f"vaug{i}") for i in range(NT)]
            aoT = [persist.tile([128, S], BF16, tag=f"aoT{i}", name=f"aoT{i}")
                   for i in range(4)]
            # ones columns of v_aug written once (denominator -> po row 64)
            for it in range(NT):
                nc.gpsimd.memset(v_aug[it][:, 64:65], 1.0)
                nc.gpsimd.memset(v_aug[it][:, 129:130], 1.0)

            def rope_tile(ps, dst, dst_cols, ccols):
                """RoPE a [128, 1024] fp32 psum tile -> dst[:, dst_cols] bf16.

                2 head blocks of 64 rows, head_dim permuted even-first.
                Evict to bf16 on Act, then 6 bf16 DVE ops (2x mode).
                """
                t0 = ropep.tile([128, 1024], BF16, tag="t0")
                nc.scalar.activation(t0[:], ps[:], COPYF)
                cos_c = cosF[:, ccols]
                sin_c = sinF[:, ccols]
                t1 = ropep.tile([128, 1024], BF16, tag="t1")
                for o in (0, 64):
                    # t1[even] = t0[odd] * (-sin); t1[odd] = t0[even] * (+sin)
                    # sinF rows: [+sin, -sin, +sin, -sin] so in0/in1 bases match
                    nc.vector.tensor_mul(t1[o:o + 32, :], t0[o + 32:o + 64, :],
                                         sin_c[o + 32:o + 64, :])
                    nc.vector.tensor_mul(t1[o + 32:o + 64, :], t0[o:o + 32, :],
                                         sin_c[o:o + 32, :])
                t2 = ropep.tile([128, 1024], BF16, tag="t2")
                nc.vector.tensor_mul(t2[:], t0[:], cos_c[:])
                nc.vector.tensor_add(dst[:, dst_cols], t2[:], t1[:])

            # ---------- filler machinery ----------
            # Units are callables emitting a short burst of independent PE
            # work; popped into the attention kt-loop to keep the PE dense.
            filler = []

            def pop_filler(k=1):
                for _ in range(k):
                    if filler:
                        filler.pop(0)()

            def make_qkv_units(C):
                """QKV + rope for 1024-token chunk C as filler units."""
                ccols = slice(1024 * C, 1024 * (C + 1))
                units = []
                for et in range(6):
                    # psq allocated lazily inside the first unit of the group
                    box = {}

                    def mk_mm(et, h, dts, box):
                        def u():
                            if "ps" not in box:
                                box["ps"] = ps_q.tile(
                                    [128, 1024], F32, tag="psq",
                                    name=f"psq_{C}_{et}")
                            ps = box["ps"]
                            col = slice(512 * (2 * C + h),
                                        512 * (2 * C + h + 1))
                            for dt in dts:
                                nc.tensor.matmul(
                                    ps[:, 512 * h:512 * (h + 1)],
                                    wq_sb[dt][:, 128 * et:128 * (et + 1)],
                                    xT[dt][:, col],
                                    start=(dt == 0), stop=(dt == 15))
                        return u

                    def mk_fin(et, box):
                        def u():
                            ps = box["ps"]
                            if et < 4:
                                rope_tile(ps, qT[et], ccols, ccols)
                            elif et == 4:
                                rope_tile(ps, kT, ccols, ccols)
                            else:
                                vt = ropep.tile([128, 1024], BF16, tag="t0")
                                nc.scalar.activation(vt[:], ps[:], COPYF)
                                for tt in range(8):
                                    it = 8 * C + tt
                                    pt = ps_y.tile([128, 128], BF16, tag="pt")
                                    nc.tensor.transpose(
                                        pt[:], vt[:, 128 * tt:128 * (tt + 1)],
                                        ident[:])
                                    nc.vector.tensor_copy(
                                        v_aug[it][:, 0:64], pt[:, 0:64])
                                    nc.vector.tensor_copy(
                                        v_aug[it][:, 65:129], pt[:, 64:128])
                        return u

                    for h in range(2):
                        for d0 in range(0, 16, 8):
                            units.append(mk_mm(et, h, range(d0, d0 + 8), box))
                    units.append(mk_fin(et, box))
                return units

            def make_staged_units(c, tail=False):
                """Output projection for 512-token subchunk c as units.

                tail=True alternates PSUM between the psy pool and the (then
                idle) po pool so the final drain double-buffers.
                """
                units = []
                for tt in range(4):
                    trow = slice(512 * c + 128 * tt, 512 * c + 128 * (tt + 1))
                    for ec in range(4):
                        def u(trow=trow, ec=ec, k=4 * tt + ec):
                            if tail and k % 2:
                                psy = ps_o.tile([128, 512], F32, tag="po",
                                                name=f"psyo_{c}_{k}")
                            else:
                                psy = ps_y.tile([128, 512], F32, tag="psy",
                                                name=f"psy_{c}_{k}")
                            for dt in range(4):
                                nc.tensor.matmul(
                                    psy[:], aoT[dt][:, trow],
                                    wo_bf[dt][:, 512 * ec:512 * (ec + 1)],
                                    start=(dt == 0), stop=(dt == 3))
                            ysb = ysbp.tile([128, 512], F32, tag="ysb")
                            nc.vector.tensor_copy(ysb[:], psy[:])
                            nc.sync.dma_start(
                                out_d[trow, 512 * ec:512 * (ec + 1)], ysb[:])
                        units.append(u)
                return units

            def attention_subchunk(c):
                """Attention for 512-token subchunk c (q cols qcol).

                h2 innermost: the h2=1 score stationary occupies PE row
                group 64:127 (tile_position inferred from base partition),
                so its LDWEIGHTS pulls ahead during the h2=0 matmul.
                """
                qcol0 = 512 * c
                n_tk = 4 * c + 4
                for m in range(4):
                    po = [ps_o.tile([65, 512], F32, tag="po",
                                    name=f"po_{c}_{m}_{h2}")
                          for h2 in range(2)]
                    for kt in range(n_tk):
                        r = kt - 4 * c
                        nq = 512 if r < 0 else 512 - 128 * r
                        qoff = qcol0 + (512 - nq)
                        pbf = [None, None]
                        for h2 in range(2):
                            o = 64 * h2
                            ss = ps_s.tile([128, 512], F32, tag="ss")
                            nc.tensor.matmul(
                                ss[:, 0:nq],
                                kT[o:o + 64, 128 * kt:128 * (kt + 1)],
                                qT[m][o:o + 64, qoff:qoff + nq],
                                start=True, stop=True)
                            pb = pbfp.tile([128, 512], BF16, tag="pbf")
                            nc.scalar.activation(pb[:, 0:nq], ss[:, 0:nq],
                                                 EXP, scale=SCALE)
                            if r >= 0:
                                nc.vector.tensor_mul(
                                    pb[:, 0:128], pb[:, 0:128], tri[:])
                            pbf[h2] = pb
                        for h2 in range(2):
                            nc.tensor.matmul(
                                po[h2][:, 512 - nq:512],
                                v_aug[kt][:, 65 * h2:65 * h2 + 65],
                                pbf[h2][:, 0:nq],
                                start=(kt == 0), stop=(kt == n_tk - 1),
                                skip_group_check=True)
                        pop_filler(1)
                    for h2 in range(2):
                        o = 64 * h2
                        # stage denominator to SBUF partition 0: the custom
                        # recip op drops nonzero base partitions, and PSUM
                        # reads must be partition-aligned
                        dnc = dnp.tile([1, 512], F32, tag="dnc")
                        nc.vector.tensor_copy(dnc[:], po[h2][64:65, :])
                        dnr = dnp.tile([1, 512], F32, tag="dnr")
                        nc.vector.reciprocal_approx_fast(
                            out=dnr[:], in_=dnc[:])
                        rb = rbp.tile([64, 512], F32, tag="rb")
                        nc.gpsimd.partition_broadcast(rb[:], dnr[:])
                        nc.vector.tensor_mul(
                            aoT[m][o:o + 64, qcol0:qcol0 + 512],
                            po[h2][0:64, :], rb[:])
                        if DBG and c == 0:
                            nc.sync.dma_start(
                                dbg_dn[2 * m + h2:2 * m + h2 + 1, :], dnr[:])
                        pop_filler(1)

            # ---------- schedule ----------
            # chunk 0 QKV runs dense (HAM warm-up), chunk 1 QKV + stage D
            # interleave into the attention kt-loops.
            for u in make_qkv_units(0):
                u()
            filler.extend(make_qkv_units(1))
            for c in range(NSC):
                if c == 2:
                    # chunk-1 qT/kT/v must exist before subchunk 2 reads them
                    while filler:
                        pop_filler(1)
                attention_subchunk(c)
                if c > 0:
                    filler.extend(make_staged_units(c - 1))
            # drain remaining filler + last subchunk's projection
            while filler:
                pop_filler(1)
            for u in make_staged_units(NSC - 1, tail=True):
                u()
            if DBG:
                nc.sync.dma_start(dbg_q[:], qT[0][:])
                nc.sync.dma_start(dbg_k[:], kT[:])
                nc.sync.dma_start(dbg_v[:], v_aug[0][:])
                nc.sync.dma_start(dbg_ao[:], aoT[0][:])

    nc.compile()
    return nc


# Local Q heads are processed in pairs (l, l+4): pair tile m holds head l
# at rows 0:64 (kv j=0) and head l+4 at rows 64:128 (kv j=1).
HEAD_ORDER = [0, 4, 1, 5, 2, 6, 3, 7]


def _to_bf16(a):
    import ml_dtypes
    return np.ascontiguousarray(a.astype(ml_dtypes.bfloat16))


def _prep_inputs(x, freqs_cis, wqkv, wo):
    """Host-side sharding: returns list of 8 in_maps."""
    perm = np.concatenate([np.arange(0, HD, 2), np.arange(1, HD, 2)])
    cos = np.ascontiguousarray(freqs_cis[:, :, 0].T.astype(np.float32))  # [32,S]
    sin = np.ascontiguousarray(freqs_cis[:, :, 1].T.astype(np.float32))
    rope = _to_bf16(
        np.concatenate([cos, cos, cos, cos,
                        sin, -sin, sin, -sin], axis=0))  # [256,S]
    xT_by_b = [_to_bf16(x[b].T) for b in range(B)]
    in_maps = []
    for c in range(8):
        b, g = c // 4, c % 4
        # [HL, HD, D] with head_dim even-first permutation + head pairing
        wq_rows = wqkv[EQ * g:EQ * (g + 1)].reshape(HL, HD, D)[:, perm, :]
        wq_rows = wq_rows[HEAD_ORDER].reshape(EQ, D)
        wk_rows = wqkv[D + EK * g:D + EK * (g + 1)].reshape(
            KVL, HD, D)[:, perm, :].reshape(EK, D)
        wv_rows = wqkv[D + NKV * HD + EV * g:D + NKV * HD + EV * (g + 1)]
        wq_cat = np.concatenate([wq_rows, wk_rows, wv_rows], axis=0)
        # woT rows reordered to the paired-head d-block layout
        woT = wo[:, EQ * g:EQ * (g + 1)].T.reshape(HL, HD, D)
        woT = woT[HEAD_ORDER].reshape(EQ, D)
        in_maps.append({
            "x": xT_by_b[b],
            "wq": _to_bf16(wq_cat.T),
            "wo": _to_bf16(woT),
            "rope": rope,
        })
    return in_maps


def _get_nc():
    global _CACHED_NC
    if _CACHED_NC is None:
        _CACHED_NC = build()
    return _CACHED_NC


def kernel(x, freqs_cis, wqkv, wo, _trace=False, _trace_kwargs=None):
    nc = _get_nc()
    in_maps = _prep_inputs(x, freqs_cis, wqkv, wo)
    res = bass_utils.run_bass_kernel_spmd(
        nc, in_maps, core_ids=list(range(8)), trace=_trace,
        **(_trace_kwargs or {}))
    outs = [res.results[c]["out"] for c in range(8)]
    y = np.stack([
        outs[0] + outs[1] + outs[2] + outs[3],
        outs[4] + outs[5] + outs[6] + outs[7],
    ]).astype(np.float32)
    kernel.last_results = res
    return y


# revision 33
# speedup vs baseline: 1.0091x; 1.0091x over previous
"""Distributed Trainium2 Bass kernel for GQA causal attention with RoPE.

Problem: B=2, S=2048, DIM=2048, 32 Q heads, 8 KV heads (GQA 4:1), hd=64,
causal, rotary embeddings, fp32 in/out.

Sharding over 8 cores: data-parallel over batch (2) x tensor-parallel over
KV-head groups (4 groups of 2 KV heads, each with its 8 Q heads).
Core c: batch b = c // 4, group g = c % 4.  Each core computes a partial
output projection; the host sums the 4 partials per batch.

v2 design (vs baseline): everything except PSUM accumulation runs in bf16
(x/wq/wo cast host-side).  QKV is computed per 1024-token chunk; attention
per 512-token subchunk with fine-grained causal narrowing on the diagonal
128-tiles.  The PE instruction stream is kept dense by interleaving
independent matmuls (output projection of the previous subchunk, QKV of the
next chunk) into the Act-coupled attention loop, which keeps the HAM clock
gate at full rate.  Causal masking is a multiplicative bf16 lower-tri mask
on the DVE (post-exp); softmax denominators come free from a ones-column in
the V operand and are inverted per-block with reciprocal_approx_fast.

Layout tricks (kept from baseline):
- head_dim permuted even-first (via Wq/Wk row permutation) so RoPE becomes
  32-row block ops; local Q heads paired (l, l+4) per 128-row tile so the
  natural K tile provides the score stationary operand for both pair
  members; woT rows reordered to match.
"""
import os
import sys

if "/opt/trn_rl_repo" not in sys.path:
    sys.path.insert(0, "/opt/trn_rl_repo")

import contextlib

import numpy as np

import concourse.bass as bass
import concourse.tile as tile
from concourse import bacc, mybir
from concourse import bass_utils
from concourse.masks import make_identity

F32 = mybir.dt.float32
BF16 = mybir.dt.bfloat16
EXP = mybir.ActivationFunctionType.Exp
COPYF = mybir.ActivationFunctionType.Copy

B, S, D = 2, 2048, 2048
NH, NKV, HD = 32, 8, 64
HL = 8           # local Q heads per core
KVL = 2          # local KV heads per core
EQ = HL * HD     # 512 local q features
EK = KVL * HD    # 128
EV = KVL * HD    # 128
EQKV = EQ + EK + EV  # 768
NT = S // 128    # 16 token tiles
NSC = S // 512   # 4 subchunks (attention granularity)
NC = S // 1024   # 2 chunks (QKV granularity)
SCALE = 1.0 / 8.0

_CACHED_NC = None


def build():
    nc = bacc.Bacc("TRN2", target_bir_lowering=False, debug=False)
    # x is host-transposed: [D, S], bf16
    x_d = nc.dram_tensor("x", [D, S], BF16, kind="ExternalInput").ap()
    wq_d = nc.dram_tensor("wq", [D, EQKV], BF16, kind="ExternalInput").ap()
    wo_d = nc.dram_tensor("wo", [EQ, D], BF16, kind="ExternalInput").ap()
    # rope rows: 0:128 = cos x4, 128:256 = [+sin, -sin, +sin, -sin]
    # (sin signs placed so each t1 mul reads t0 and sin at the SAME base
    # partition -- walrus requires equal base partitions for two SB inputs)
    rope_d = nc.dram_tensor("rope", [256, S], BF16, kind="ExternalInput").ap()
    out_d = nc.dram_tensor("out", [S, D], F32, kind="ExternalOutput").ap()
    DBG = bool(int(os.environ.get("K_DEBUG", "0")))
    if DBG:
        dbg_q = nc.dram_tensor("dbg_q", [128, S], BF16,
                               kind="ExternalOutput").ap()
        dbg_k = nc.dram_tensor("dbg_k", [128, S], BF16,
                               kind="ExternalOutput").ap()
        dbg_v = nc.dram_tensor("dbg_v", [128, 130], BF16,
                               kind="ExternalOutput").ap()
        dbg_ao = nc.dram_tensor("dbg_ao", [128, S], BF16,
                                kind="ExternalOutput").ap()
        dbg_dn = nc.dram_tensor("dbg_dn", [8, 512], F32,
                                kind="ExternalOutput").ap()

    with tile.TileContext(nc) as tc:
        ctx = contextlib.ExitStack()
        with ctx:
            const = ctx.enter_context(tc.tile_pool(name="const", bufs=1))
            persist = ctx.enter_context(tc.tile_pool(name="persist", bufs=1))
            xtp = ctx.enter_context(tc.tile_pool(name="xt", bufs=16))
            ropep = ctx.enter_context(tc.tile_pool(name="ropep", bufs=3))
            pbfp = ctx.enter_context(tc.tile_pool(name="pbf", bufs=4))
            dnp = ctx.enter_context(tc.tile_pool(name="dnp", bufs=3))
            rbp = ctx.enter_context(tc.tile_pool(name="rbp", bufs=3))
            ysbp = ctx.enter_context(tc.tile_pool(name="ysb", bufs=3))
            # PSUM (8 banks): psq 1x2, ss 2x1, po 2x1, psy 1x(psy+pt) = 8
            ps_q = ctx.enter_context(
                tc.tile_pool(name="psq", bufs=1, space="PSUM"))
            ps_s = ctx.enter_context(
                tc.tile_pool(name="pss", bufs=2, space="PSUM"))
            ps_o = ctx.enter_context(
                tc.tile_pool(name="pso", bufs=2, space="PSUM"))
            ps_y = ctx.enter_context(
                tc.tile_pool(name="psy", bufs=1, space="PSUM"))

            # ---- constants ----
            ident = const.tile([128, 128], BF16)
            make_identity(nc, ident[:])
            # lower-tri (keep q>=k) multiplicative mask: tri[p,j]=1 iff j>=p
            ones_t = const.tile([128, 128], BF16, name="ones_t")
            nc.gpsimd.memset(ones_t[:], 1.0)
            tri = const.tile([128, 128], BF16, name="tri")
            nc.gpsimd.affine_select(
                out=tri[:], in_=ones_t[:],
                compare_op=mybir.AluOpType.is_ge,
                fill=0.0, base=0, channel_multiplier=-1,
                pattern=[[1, 128]])

            # ---- weights + x loads, issue-interleaved so the first QKV
            # dt-chain starts as early as possible (SP issues ~645ns/DMA) ----
            wq_sb, xT = [], []
            for dt in range(16):
                wb = const.tile([128, EQKV], BF16, tag=f"wqb{dt}",
                                name=f"wqb{dt}")
                wq_sb.append(wb)
                xt = xtp.tile([128, S], BF16, tag="xt", name=f"xT{dt}")
                xT.append(xt)
            # issue dt 0-7 pairs on SP, dt 8-15 pairs on the Act DGE queue
            # concurrently (issue rate ~650ns/DMA gates the first QKV sweep)
            for dt in range(8):
                for eng, d0 in ((nc.sync, dt), (nc.scalar, dt + 8)):
                    eng.dma_start(wq_sb[d0][:],
                                  wq_d[128 * d0:128 * (d0 + 1), :])
                    eng.dma_start(xT[d0][:], x_d[128 * d0:128 * (d0 + 1), :])
            cosF = const.tile([128, S], BF16)
            nc.sync.dma_start(cosF[:], rope_d[0:128, :])
            sinF = const.tile([128, S], BF16)
            nc.sync.dma_start(sinF[:], rope_d[128:256, :])
            wo_bf = []
            for dt in range(4):
                wb = const.tile([128, D], BF16, tag=f"wob{dt}", name=f"wob{dt}")
                nc.sync.dma_start(wb[:], wo_d[128 * dt:128 * (dt + 1), :])
                wo_bf.append(wb)

            # ---- persistent activation buffers ----
            qT = [persist.tile([128, S], BF16, tag=f"qT{i}", name=f"qT{i}")
                  for i in range(4)]
            kT = persist.tile([128, S], BF16, name="kT")
            v_aug = [persist.tile([128, 130], BF16, tag=f"vaug{i}",
                                  name=f"vaug{i}") for i in range(NT)]
            aoT = [persist.tile([128, S], BF16, tag=f"aoT{i}", name=f"aoT{i}")
                   for i in range(4)]
            # ones columns of v_aug written once (denominator -> po row 64)
            for it in range(NT):
                nc.gpsimd.memset(v_aug[it][:, 64:65], 1.0)
                nc.gpsimd.memset(v_aug[it][:, 129:130], 1.0)

            def evict_1024(halves, tag="t0"):
                """Evict one or two [128, 512] fp32 psum APs -> bf16 sbuf."""
                t0 = ropep.tile([128, 1024], BF16, tag=tag)
                if len(halves) == 1:
                    nc.scalar.activation(t0[:], halves[0], COPYF)
                else:
                    for i, h in enumerate(halves):
                        nc.scalar.activation(
                            t0[:, 512 * i:512 * (i + 1)], h, COPYF)
                return t0

            def rope_tile(ps_halves, dst, dst_cols, ccols):
                """RoPE [128, 1024] fp32 psum (1-2 pieces) -> dst bf16.

                2 head blocks of 64 rows, head_dim permuted even-first.
                Evict to bf16 on Act, then 6 bf16 DVE ops (2x mode).
                """
                t0 = evict_1024(ps_halves)
                cos_c = cosF[:, ccols]
                sin_c = sinF[:, ccols]
                t1 = ropep.tile([128, 1024], BF16, tag="t1")
                for o in (0, 64):
                    # t1[even] = t0[odd] * (-sin); t1[odd] = t0[even] * (+sin)
                    # sinF rows: [+sin, -sin, +sin, -sin] so in0/in1 bases match
                    nc.vector.tensor_mul(t1[o:o + 32, :], t0[o + 32:o + 64, :],
                                         sin_c[o + 32:o + 64, :])
                    nc.vector.tensor_mul(t1[o + 32:o + 64, :], t0[o:o + 32, :],
                                         sin_c[o:o + 32, :])
                t2 = ropep.tile([128, 1024], BF16, tag="t2")
                nc.vector.tensor_mul(t2[:], t0[:], cos_c[:])
                nc.vector.tensor_add(dst[:, dst_cols], t2[:], t1[:])

            # ---------- filler machinery ----------
            # Units are callables emitting a short burst of independent PE
            # work; popped into the attention kt-loop to keep the PE dense.
            filler = []

            def pop_filler(k=1):
                for _ in range(k):
                    if filler:
                        filler.pop(0)()

            def qkv_fin(C, et, halves):
                """Rope/V-transpose consuming the et accumulator halves."""
                ccols = slice(1024 * C, 1024 * (C + 1))
                if et < 4:
                    rope_tile(halves, qT[et], ccols, ccols)
                elif et == 4:
                    rope_tile(halves, kT, ccols, ccols)
                else:
                    vt = evict_1024(halves)
                    for tt in range(8):
                        it = 8 * C + tt
                        pt = ps_y.tile([128, 128], BF16, tag="pt")
                        nc.tensor.transpose(
                            pt[:], vt[:, 128 * tt:128 * (tt + 1)],
                            ident[:])
                        nc.vector.tensor_copy(
                            v_aug[it][:, 0:64], pt[:, 0:64])
                        nc.vector.tensor_copy(
                            v_aug[it][:, 65:129], pt[:, 64:128])

            def make_qkv_units(C):
                """QKV + rope for 1024-token chunk C as filler units.

                Consecutive h=0/h=1 matmuls share their stationary operand
                (dt-outer, h-inner) to give walrus a LDWEIGHTS-dedup shot.
                """
                units = []
                for et in range(6):
                    box = {}

                    def mk_mm(et, dts, box):
                        def u():
                            if "ps" not in box:
                                box["ps"] = ps_q.tile(
                                    [128, 1024], F32, tag="psq",
                                    name=f"psq_{C}_{et}")
                            ps = box["ps"]
                            for dt in dts:
                                for h in range(2):
                                    col = slice(512 * (2 * C + h),
                                                512 * (2 * C + h + 1))
                                    nc.tensor.matmul(
                                        ps[:, 512 * h:512 * (h + 1)],
                                        wq_sb[dt][:, 128 * et:128 * (et + 1)],
                                        xT[dt][:, col],
                                        start=(dt == 0), stop=(dt == 15))
                        return u

                    for d0 in range(0, 16, 4):
                        units.append(mk_mm(et, range(d0, d0 + 4), box))
                    units.append(lambda et=et, box=box: qkv_fin(
                        C, et, [box["ps"][:, 0:512], box["ps"][:, 512:1024]]))
                return units

            def emit_qkv_chunk0():
                """Chunk-0 QKV, et-pipelined: even et accumulate in the psq
                pool, odd et in two borrowed ss-pool banks, so rope eviction
                of one et overlaps the next et's matmuls (and the first
                dt sweep overlaps the x/wq DMA trickle)."""
                for ep in range(3):  # et pairs (0,1), (2,3), (4,5)
                    e0, e1 = 2 * ep, 2 * ep + 1
                    pse = ps_q.tile([128, 1024], F32, tag="psq",
                                    name=f"psq0_{e0}")
                    odd = [ps_s.tile([128, 512], F32, tag="ss",
                                     name=f"ssq0_{e1}_{h}") for h in range(2)]
                    for dt in range(16):
                        for h in range(2):  # e0: h-inner shares stationary
                            nc.tensor.matmul(
                                pse[:, 512 * h:512 * (h + 1)],
                                wq_sb[dt][:, 128 * e0:128 * (e0 + 1)],
                                xT[dt][:, 512 * h:512 * (h + 1)],
                                start=(dt == 0), stop=(dt == 15))
                        for h in range(2):
                            nc.tensor.matmul(
                                odd[h][:],
                                wq_sb[dt][:, 128 * e1:128 * (e1 + 1)],
                                xT[dt][:, 512 * h:512 * (h + 1)],
                                start=(dt == 0), stop=(dt == 15))
                    qkv_fin(0, e0, [pse[:, 0:512], pse[:, 512:1024]])
                    qkv_fin(0, e1, [odd[0][:], odd[1][:]])

            def make_staged_units(c, tail=False):
                """Output projection for 512-token subchunk c as units.

                tail=True alternates PSUM between the psy pool and the (then
                idle) po pool so the final drain double-buffers.
                """
                units = []
                for tt in range(4):
                    trow = slice(512 * c + 128 * tt, 512 * c + 128 * (tt + 1))
                    for ec in range(4):
                        def u(trow=trow, ec=ec, k=4 * tt + ec):
                            if tail and k % 2:
                                psy = ps_o.tile([128, 512], F32, tag="po",
                                                name=f"psyo_{c}_{k}")
                            else:
                                psy = ps_y.tile([128, 512], F32, tag="psy",
                                                name=f"psy_{c}_{k}")
                            for dt in range(4):
                                nc.tensor.matmul(
                                    psy[:], aoT[dt][:, trow],
                                    wo_bf[dt][:, 512 * ec:512 * (ec + 1)],
                                    start=(dt == 0), stop=(dt == 3))
                            ysb = ysbp.tile([128, 512], F32, tag="ysb")
                            nc.vector.tensor_copy(ysb[:], psy[:])
                            nc.sync.dma_start(
                                out_d[trow, 512 * ec:512 * (ec + 1)], ysb[:])
                        units.append(u)
                return units

            def attention_subchunk(c):
                """Attention for 512-token subchunk c (q cols qcol).

                h2 innermost: the h2=1 score stationary occupies PE row
                group 64:127 (tile_position inferred from base partition),
                so its LDWEIGHTS pulls ahead during the h2=0 matmul.
                """
                qcol0 = 512 * c
                n_tk = 4 * c + 4
                for m in range(4):
                    po = [ps_o.tile([65, 512], F32, tag="po",
                                    name=f"po_{c}_{m}_{h2}")
                          for h2 in range(2)]
                    for kt in range(n_tk):
                        r = kt - 4 * c
                        nq = 512 if r < 0 else 512 - 128 * r
                        qoff = qcol0 + (512 - nq)
                        pbf = [None, None]
                        for h2 in range(2):
                            o = 64 * h2
                            ss = ps_s.tile([128, 512], F32, tag="ss")
                            nc.tensor.matmul(
                                ss[:, 0:nq],
                                kT[o:o + 64, 128 * kt:128 * (kt + 1)],
                                qT[m][o:o + 64, qoff:qoff + nq],
                                start=True, stop=True)
                            pb = pbfp.tile([128, 512], BF16, tag="pbf")
                            nc.scalar.activation(pb[:, 0:nq], ss[:, 0:nq],
                                                 EXP, scale=SCALE)
                            if r >= 0:
                                nc.vector.tensor_mul(
                                    pb[:, 0:128], pb[:, 0:128], tri[:])
                            pbf[h2] = pb
                        for h2 in range(2):
                            nc.tensor.matmul(
                                po[h2][:, 512 - nq:512],
                                v_aug[kt][:, 65 * h2:65 * h2 + 65],
                                pbf[h2][:, 0:nq],
                                start=(kt == 0), stop=(kt == n_tk - 1),
                                skip_group_check=True)
                        pop_filler(1)
                    for h2 in range(2):
                        o = 64 * h2
                        # stage denominator to SBUF partition 0: the custom
                        # recip op drops nonzero base partitions, and PSUM
                        # reads must be partition-aligned
                        dnc = dnp.tile([1, 512], F32, tag="dnc")
                        nc.vector.tensor_copy(dnc[:], po[h2][64:65, :])
                        dnr = dnp.tile([1, 512], F32, tag="dnr")
                        nc.vector.reciprocal_approx_fast(
                            out=dnr[:], in_=dnc[:])
                        rb = rbp.tile([64, 512], F32, tag="rb")
                        nc.gpsimd.partition_broadcast(rb[:], dnr[:])
                        nc.vector.tensor_mul(
                            aoT[m][o:o + 64, qcol0:qcol0 + 512],
                            po[h2][0:64, :], rb[:])
                        if DBG and c == 0:
                            nc.sync.dma_start(
                                dbg_dn[2 * m + h2:2 * m + h2 + 1, :], dnr[:])
                        pop_filler(1)

            # ---------- schedule ----------
            # chunk 0 QKV runs dense (HAM warm-up), chunk 1 QKV + stage D
            # interleave into the attention kt-loops.
            emit_qkv_chunk0()
            filler.extend(make_qkv_units(1))
            for c in range(NSC):
                if c == 2:
                    # chunk-1 qT/kT/v must exist before subchunk 2 reads them
                    while filler:
                        pop_filler(1)
                attention_subchunk(c)
                if c > 0:
                    filler.extend(make_staged_units(c - 1))
            # drain remaining filler + last subchunk's projection.
            # Tail uses ec-pairs with dt-outer (stationary shared by the
            # pair) across the now-free psy+po pools so evicts pipeline.
            while filler:
                pop_filler(1)
            c = NSC - 1
            for tt in range(4):
                trow = slice(512 * c + 128 * tt, 512 * c + 128 * (tt + 1))
                for e0 in (0, 2):
                    psy2 = [ps_y.tile([128, 512], F32, tag="psy",
                                      name=f"tl_{tt}_{e0}"),
                            ps_o.tile([128, 512], F32, tag="po",
                                      name=f"tlo_{tt}_{e0}")]
                    for dt in range(4):
                        for i, ec in enumerate((e0, e0 + 1)):
                            nc.tensor.matmul(
                                psy2[i][:], aoT[dt][:, trow],
                                wo_bf[dt][:, 512 * ec:512 * (ec + 1)],
                                start=(dt == 0), stop=(dt == 3))
                    for i, ec in enumerate((e0, e0 + 1)):
                        ysb = ysbp.tile([128, 512], F32, tag="ysb")
                        nc.vector.tensor_copy(ysb[:], psy2[i][:])
                        nc.sync.dma_start(
                            out_d[trow, 512 * ec:512 * (ec + 1)], ysb[:])
            if DBG:
                nc.sync.dma_start(dbg_q[:], qT[0][:])
                nc.sync.dma_start(dbg_k[:], kT[:])
                nc.sync.dma_start(dbg_v[:], v_aug[0][:])
                nc.sync.dma_start(dbg_ao[:], aoT[0][:])

    nc.compile()
    return nc


# Local Q heads are processed in pairs (l, l+4): pair tile m holds head l
# at rows 0:64 (kv j=0) and head l+4 at rows 64:128 (kv j=1).
HEAD_ORDER = [0, 4, 1, 5, 2, 6, 3, 7]


def _to_bf16(a):
    import ml_dtypes
    return np.ascontiguousarray(a.astype(ml_dtypes.bfloat16))


def _prep_inputs(x, freqs_cis, wqkv, wo):
    """Host-side sharding: returns list of 8 in_maps."""
    perm = np.concatenate([np.arange(0, HD, 2), np.arange(1, HD, 2)])
    cos = np.ascontiguousarray(freqs_cis[:, :, 0].T.astype(np.float32))  # [32,S]
    sin = np.ascontiguousarray(freqs_cis[:, :, 1].T.astype(np.float32))
    rope = _to_bf16(
        np.concatenate([cos, cos, cos, cos,
                        sin, -sin, sin, -sin], axis=0))  # [256,S]
    xT_by_b = [_to_bf16(x[b].T) for b in range(B)]
    in_maps = []
    for c in range(8):
        b, g = c // 4, c % 4
        # [HL, HD, D] with head_dim even-first permutation + head pairing
        wq_rows = wqkv[EQ * g:EQ * (g + 1)].reshape(HL, HD, D)[:, perm, :]
        wq_rows = wq_rows[HEAD_ORDER].reshape(EQ, D)
        wk_rows = wqkv[D + EK * g:D + EK * (g + 1)].reshape(
            KVL, HD, D)[:, perm, :].reshape(EK, D)
        wv_rows = wqkv[D + NKV * HD + EV * g:D + NKV * HD + EV * (g + 1)]
        wq_cat = np.concatenate([wq_rows, wk_rows, wv_rows], axis=0)
        # woT rows reordered to the paired-head d-block layout
        woT = wo[:, EQ * g:EQ * (g + 1)].T.reshape(HL, HD, D)
        woT = woT[HEAD_ORDER].reshape(EQ, D)
        in_maps.append({
            "x": xT_by_b[b],
            "wq": _to_bf16(wq_cat.T),
            "wo": _to_bf16(woT),
            "rope": rope,
        })
    return in_maps


def _get_nc():
    global _CACHED_NC
    if _CACHED_NC is None:
        _CACHED_NC = build()
    return _CACHED_NC


def kernel(x, freqs_cis, wqkv, wo, _trace=False, _trace_kwargs=None):
    nc = _get_nc()
    in_maps = _prep_inputs(x, freqs_cis, wqkv, wo)
    res = bass_utils.run_bass_kernel_spmd(
        nc, in_maps, core_ids=list(range(8)), trace=_trace,
        **(_trace_kwargs or {}))
    outs = [res.results[c]["out"] for c in range(8)]
    y = np.stack([
        outs[0] + outs[1] + outs[2] + outs[3],
        outs[4] + outs[5] + outs[6] + outs[7],
    ]).astype(np.float32)
    kernel.last_results = res
    return y
